# revision 13
# baseline (speedup 1.0000x reference)
"""Chamfer-KL loss kernel for Trainium2 (Bass/Tile).

Math (per batch element b):
    inner[x,y] = d + sum_la[x] - sum_lb[y] - t_var - t_mua + 2*t_cross - t_mub[y]
    p_kl = -0.5 * inner
    loss = sum_y min_x p_kl + sum_x mask[x] * min_y p_kl

We compute T = d - inner as a single K=258 GEMM:
    T[x,y] = L1.R1 + L2.R2 + L3.R3   (K blocks 128 + 128 + 2)
      L1 = (exp(la) + mu_a^2)^T        R1 = exp(-lb)^T
      L2 = (-2 mu_a)^T                 R2 = (mu_b * exp(-lb))^T
      L3 = [ones; sum_la]              R3 = [sum_lb + t_mub; -ones]
so p_kl = 0.5*(T - d), min commutes with the monotone map, and
    loss = 0.5*sum_y (min_x T - d) + 0.5*sum_x mask*(min_y T - d).

Sharding: data-parallel over batch; core i handles batch element i fully.
"""

import os
import numpy as np

import concourse.bass as bass
import concourse.tile as tile
from concourse import mybir
from concourse.bass_utils import run_bass_kernel_spmd
from concourse.masks import make_identity

F32 = mybir.dt.float32
F32R = mybir.dt.float32r
BF16 = mybir.dt.bfloat16
AX = mybir.AxisListType
OP = mybir.AluOpType
AF = mybir.ActivationFunctionType

BS, NX, NY, D = 8, 4096, 4096, 128
P = 128      # SBUF partitions
YB = 512     # y block = one PSUM bank of fp32


def _body(tc, mu_a, la, mu_b, lb, mask, out_d, nx, ny, epi_bf16):
    nc = tc.nc
    nt = nx // P     # x tiles
    nu = ny // P     # y chunks of 128
    nyb = ny // YB   # y blocks of 512
    ecast = BF16 if epi_bf16 else F32

    with tc.tile_pool(name="const", bufs=1) as const:
        ident = const.tile([P, P], F32)
        make_identity(nc, ident)
        ident_e = const.tile([P, P], ecast)
        nc.vector.tensor_copy(ident_e, ident)
        ones_f = const.tile([P, 1], F32)
        nc.vector.memset(ones_f, 1.0)
        ones_r = const.tile([P, 1], F32R)
        nc.vector.tensor_copy(ones_r, ones_f)

        L1 = const.tile([P, nx], F32R)
        L2 = const.tile([P, nx], F32R)
        R1 = const.tile([P, ny], F32R)
        R2 = const.tile([P, ny], F32R)
        combo = const.tile([P, ny], F32R)
        L3 = const.tile([2, nx], F32R)
        R3 = const.tile([2, ny], F32R)
        colmin = const.tile([P, ny], ecast)
        rowmin_all = const.tile([P, nt], F32)
        sumla_nat = const.tile([P, nt], F32)
        mask_sb = const.tile([P, nt], F32)

        nc.vector.memset(colmin, 1.0e30)
        # f32r tiles cannot be memset directly; stage through an f32 scratch.
        init_f = const.tile([2, max(nx, ny)], F32)
        # L3 row0 stays 1.0; row1 is overwritten by the sum_la DMA below.
        nc.vector.memset(init_f, 1.0)
        nc.vector.tensor_copy(L3, init_f[:, :nx])
        # R3 row1 stays -1.0; row0 is overwritten by the ones-matmul copies.
        neg_f = const.tile([2, max(nx, ny)], F32)
        nc.vector.memset(neg_f, -1.0)
        nc.vector.tensor_copy(R3, neg_f[:, :ny])

        # ---------------- Phase T: load, transpose, features ----------------
        with (
            tc.tile_pool(name="nat", bufs=8) as nat,
            tc.tile_pool(name="pst", bufs=4, space="PSUM") as pst,
            tc.tile_pool(name="sc", bufs=6) as sc,
        ):
            for t in range(nt):
                xs = slice(t * P, (t + 1) * P)
                n_la = nat.tile([P, D], F32, tag="nat")
                nc.gpsimd.dma_start(out=n_la, in_=la[xs, :])
                n_ma = nat.tile([P, D], F32, tag="nat")
                nc.gpsimd.dma_start(out=n_ma, in_=mu_a[xs, :])
                nc.vector.tensor_reduce(
                    sumla_nat[:, t : t + 1], n_la, axis=AX.X, op=OP.add
                )
                p_la = pst.tile([P, P], F32, tag="ps", bufs=4)
                nc.tensor.transpose(p_la, n_la, ident)
                p_ma = pst.tile([P, P], F32, tag="ps", bufs=4)
                nc.tensor.transpose(p_ma, n_ma, ident)
                e_t = sc.tile([P, P], F32, tag="sc")
                nc.scalar.activation(e_t, p_la, AF.Exp)
                sq_t = sc.tile([P, P], F32, tag="sc")
                nc.scalar.activation(sq_t, p_ma, AF.Square)
                nc.vector.tensor_add(L1[:, xs], e_t, sq_t)
                nc.scalar.mul(L2[:, xs], p_ma, -2.0)

            for t in range(nu):
                ysl = slice(t * P, (t + 1) * P)
                n_lb = nat.tile([P, D], F32, tag="nat")
                nc.gpsimd.dma_start(out=n_lb, in_=lb[ysl, :])
                n_mb = nat.tile([P, D], F32, tag="nat")
                nc.gpsimd.dma_start(out=n_mb, in_=mu_b[ysl, :])
                p_lb = pst.tile([P, P], F32, tag="ps", bufs=4)
                nc.tensor.transpose(p_lb, n_lb, ident)
                p_mb = pst.tile([P, P], F32, tag="ps", bufs=4)
                nc.tensor.transpose(p_mb, n_mb, ident)
                nc.scalar.activation(R1[:, ysl], p_lb, AF.Exp, scale=-1.0)
                nc.vector.tensor_mul(R2[:, ysl], p_mb, R1[:, ysl])
                m2_t = sc.tile([P, P], F32, tag="sc")
                nc.vector.tensor_mul(m2_t, p_mb, R2[:, ysl])
                nc.vector.tensor_add(combo[:, ysl], m2_t, p_lb)

            # L3 row1 = sum_la laid out along the free dim
            p_sla = pst.tile([nt, P], F32, tag="ps_s", bufs=2)
            nc.tensor.transpose(p_sla, sumla_nat, ident)
            sla_T = sc.tile([nt, P], F32R, tag="sc2")
            nc.vector.tensor_copy(sla_T, p_sla)
            nc.sync.dma_start(
                out=L3[1:2, :].rearrange("p (t f) -> p t f", t=nt),
                in_=sla_T,
            )
            # mask -> [P, nt]
            m_nat = sc.tile([nt, P], F32, tag="sc2")
            nc.gpsimd.dma_start(out=m_nat, in_=mask.rearrange("(t f) -> t f", f=P))
            p_m = pst.tile([P, nt], F32, tag="ps_s", bufs=2)
            nc.tensor.transpose(p_m, m_nat, ident[:nt, :nt])
            nc.vector.tensor_copy(mask_sb, p_m)

        # R3 row0 = sum_d combo via ones-matmuls
        with tc.tile_pool(name="pso", bufs=2, space="PSUM") as pso:
            for nblk in range(nyb):
                ysb = slice(nblk * YB, (nblk + 1) * YB)
                p_o = pso.tile([1, YB], F32, tag="po")
                nc.tensor.matmul(p_o, ones_r, combo[:, ysb], start=True, stop=True)
                nc.vector.tensor_copy(R3[0:1, ysb], p_o)

        # ---------------- Phase G: main GEMM + min epilogue ----------------
        with (
            tc.tile_pool(name="psm", bufs=8, space="PSUM") as psm,
            tc.tile_pool(name="bfp", bufs=6) as bfp,
            tc.tile_pool(name="slp", bufs=3) as slp,
        ):
            no_epi = bool(int(os.environ.get("KERN_NO_EPI", "0")))
            no_mm3 = bool(int(os.environ.get("KERN_NO_MM3", "0")))
            for t in range(nt):
                xs = slice(t * P, (t + 1) * P)
                slots = slp.tile([P, nyb], F32, tag="slots")
                for nblk in range(nyb):
                    ysb = slice(nblk * YB, (nblk + 1) * YB)
                    pm = psm.tile([P, YB], F32, tag="mm")
                    nc.tensor.matmul(pm, L1[:, xs], R1[:, ysb], start=True, stop=False)
                    if no_mm3:
                        nc.tensor.matmul(
                            pm, L2[:, xs], R2[:, ysb], start=False, stop=True
                        )
                    else:
                        nc.tensor.matmul(
                            pm, L2[:, xs], R2[:, ysb], start=False, stop=False
                        )
                        nc.tensor.matmul(
                            pm, L3[:, xs], R3[:, ysb], start=False, stop=True
                        )
                    cp = bfp.tile([P, YB], ecast, tag="cp")
                    if not bool(int(os.environ.get("KERN_NO_CP", "0"))):
                        nc.scalar.copy(cp, pm)
                    if not no_epi:
                        nc.vector.tensor_reduce(
                            slots[:, nblk : nblk + 1], cp, axis=AX.X, op=OP.min
                        )
                        nc.vector.tensor_tensor(
                            colmin[:, ysb], cp, colmin[:, ysb], op=OP.min
                        )
                if not no_epi:
                    nc.vector.tensor_reduce(
                        rowmin_all[:, t : t + 1], slots, axis=AX.X, op=OP.min
                    )

        # ---------------- Phase F: final reductions ----------------
        if no_epi:
            with tc.tile_pool(name="fin0", bufs=1) as fin0:
                o_sb = fin0.tile([1, 1], F32)
                nc.vector.memset(o_sb, 0.0)
                nc.sync.dma_start(out=out_d, in_=o_sb)
            return
        with (
            tc.tile_pool(name="psf", bufs=4, space="PSUM") as psf,
            tc.tile_pool(name="fin", bufs=1) as fin,
        ):
            colmin_f = fin.tile([P, nu], F32)
            for c in range(nu):
                pc = psf.tile([P, P], ecast, tag="pf", bufs=4)
                nc.tensor.transpose(pc, colmin[:, c * P : (c + 1) * P], ident_e)
                nc.vector.tensor_reduce(
                    colmin_f[:, c : c + 1], pc, axis=AX.X, op=OP.min
                )
            t1 = fin.tile([P, nu], F32)
            nc.vector.tensor_scalar_add(t1, colmin_f, -float(D))
            l1v = fin.tile([P, 1], F32)
            nc.vector.tensor_reduce(l1v, t1, axis=AX.X, op=OP.add)
            t2 = fin.tile([P, nt], F32)
            nc.vector.tensor_scalar_add(t2, rowmin_all, -float(D))
            t3 = fin.tile([P, nt], F32)
            nc.vector.tensor_mul(t3, t2, mask_sb)
            l2v = fin.tile([P, 1], F32)
            nc.vector.tensor_reduce(l2v, t3, axis=AX.X, op=OP.add)
            lv = fin.tile([P, 1], F32)
            nc.vector.tensor_add(lv, l1v, l2v)
            lv2 = fin.tile([P, 1], F32)
            nc.vector.tensor_scalar_mul(lv2, lv, 0.5)
            p11 = psf.tile([1, 1], F32, tag="p11", bufs=1)
            nc.tensor.matmul(p11, lv2, ones_f, start=True, stop=True)
            o_sb = fin.tile([1, 1], F32)
            nc.vector.tensor_copy(o_sb, p11)
            nc.sync.dma_start(out=out_d, in_=o_sb)


def _split_waits(nc, limit=1):
    """Hoist excess semaphore waits onto preceding same-engine NoOps.

    The walrus build in this container only supports a small number of sync
    wait commands per hardware instruction (PE self-loading matmuls take just
    one), while Tile freely attaches several.  Equivalent semantics: carriers
    block the engine queue before the instruction executes.
    """
    n = 0
    for f in nc.m.functions:
        for bb in f.blocks:
            insts = list(bb.instructions)
            out = []
            changed = False
            for inst in insts:
                si = inst.sync_info
                waits = list(si.on_wait) if (si is not None and si.on_wait) else []
                if len(waits) > limit:
                    for w in waits[:-limit]:
                        n += 1
                        out.append(
                            mybir.InstNoOp(
                                name=f"wsplit-{n}",
                                engine=inst.engine,
                                ins=[],
                                outs=[],
                                sync_info=mybir.SyncInfo(on_wait=[w], on_update=[]),
                            )
                        )
                    si.on_wait = waits[-limit:]
                    changed = True
                out.append(inst)
            if changed:
                bb.instructions = out
    return nc


def build(nx=NX, ny=NY, epi_bf16=True, num_devices=BS):
    nc = bass.Bass(
        "TRN2", target_bir_lowering=False, debug=False, num_devices=num_devices
    )
    mu_a = nc.dram_tensor("mu_preds", [nx, D], F32, kind="ExternalInput").ap()
    la = nc.dram_tensor("logvar_preds", [nx, D], F32, kind="ExternalInput").ap()
    mu_b = nc.dram_tensor("mu_gts", [ny, D], F32, kind="ExternalInput").ap()
    lb = nc.dram_tensor("logvar_gts", [ny, D], F32, kind="ExternalInput").ap()
    mask = nc.dram_tensor("posterior_mask", [nx], F32, kind="ExternalInput").ap()
    out_d = nc.dram_tensor("loss", [1, 1], F32, kind="ExternalOutput").ap()
    with tile.TileContext(nc) as tc:
        _body(tc, mu_a, la, mu_b, lb, mask, out_d, nx, ny, epi_bf16)
    _split_waits(nc)
    return nc


_NC_CACHE = {}


def _get_nc():
    key = "full"
    if key not in _NC_CACHE:
        _NC_CACHE[key] = build()
    return _NC_CACHE[key]


def kernel_with_stats(trace=False, **inputs):
    nc = _get_nc()
    names = ["mu_preds", "logvar_preds", "mu_gts", "logvar_gts", "posterior_mask"]
    in_maps = [
        {n: np.ascontiguousarray(inputs[n][i], dtype=np.float32) for n in names}
        for i in range(BS)
    ]
    res = run_bass_kernel_spmd(nc, in_maps, core_ids=list(range(BS)), trace=trace)
    out = np.array([res.results[i]["loss"][0, 0] for i in range(BS)], dtype=np.float32)
    return out, res


def kernel(**inputs):
    trace = bool(int(os.environ.get("KERNEL_TRACE", "0")))
    out, _ = kernel_with_stats(trace=trace, **inputs)
    return out
